# revision 11
# baseline (speedup 1.0000x reference)
"""Trainium2 Bass kernel for nn_Net_13743895347756 (gnn_message_passing).

Strategy (data-parallel over graphs, 8 cores, 32 graphs each):
  - All gather/scatter graph ops become one-hot matmuls on the PE with
    host-built 0/1 tensors (G for src-gather, D for dst-scatter).
  - Per NNConv layer the per-edge weight matrices are never materialized.
    With z[e] = h2[e] (x) h_src[e] (outer product, built on the DVE) and
    w3flat = reshape(w3, [64*din, dout]):
        agg = scatter_dst(z_ext) @ w3ext      (z_ext = [z | h_src] for b3)
    computed as  ST[c, n] = sum_e z_ext[e, c] * D[e, n]   (PE, stage 1)
                 aggT[o, n] += w3ext[c, o]^T ST[c, n]      (PE, stage 2)
    with the root term folded in as one extra accumulating matmul
    (lhsT=root, rhs=hT) and the conv bias applied on the ACT relu.
  - Node state kept transposed (hT [dout, n]); untransposed blocks for the
    gather are produced by PE transposes.
  - Weights are host-cast to bf16 and host-laid-out exactly as SBUF wants
    them, so every DMA is a plain [128, X] copy.
"""

import numpy as np
import ml_dtypes

NCORES = 8
NUM_GRAPHS = 256
NPG = 10              # nodes per graph
EPG = 16              # edges per graph
GPC = NUM_GRAPHS // NCORES   # graphs per core = 32
NPC = GPC * NPG       # nodes per core = 320
EPC = GPC * EPG       # edges per core = 512
ET = 4                # edge tiles per core (128 edges each)
EB = EPC // ET        # 128 edges per tile
NB = NPC // ET        # 80 nodes per tile-block
K = 64                # edge-MLP hidden width
DIMS = [(4, 64), (64, 128), (128, 128), (128, 64)]

BF = ml_dtypes.bfloat16
F32 = np.float32


def _bf(a):
    return np.ascontiguousarray(np.asarray(a, F32).astype(BF))


def _host_prep(x, edge_index, edge_attr1, edge_attr2, batch, params):
    """Build per-core input maps. All device tensors are laid out exactly as
    their SBUF destination tiles."""
    x = np.asarray(x, F32)
    src = np.asarray(edge_index[0]).astype(np.int64)
    dst = np.asarray(edge_index[1]).astype(np.int64)
    eattrs = [np.asarray(edge_attr1, F32), np.asarray(edge_attr2, F32)]

    # --- shared (replicated) weight tensors -------------------------------
    shared = {}
    for b in range(2):
        branch = params[f'branch{b + 1}']
        for l, (din, dout) in enumerate(DIMS):
            cp = branch[l]
            mp = cp['mlp']
            tag = f"{b}_{l}"
            w1 = np.asarray(mp['w1'], F32)            # [3, 64]
            b1 = np.asarray(mp['b1'], F32)            # [64]
            shared[f'w1e_{tag}'] = _bf(np.concatenate([w1, b1[None, :]], 0))  # [4, 64]
            w2 = np.asarray(mp['w2'], F32)            # [64, 64]
            b2 = np.asarray(mp['b2'], F32)            # [64]
            # column-duplicated so the matmul directly yields paired h2d
            w2d = np.repeat(w2, 2, axis=1)            # [64, 128]
            b2d = np.repeat(b2, 2)[None, :]           # [1, 128]
            shared[f'w2d_{tag}'] = _bf(w2d)
            shared[f'b2d_{tag}'] = _bf(b2d)
            C = K * din
            rows = C + din
            ncb = (rows + 127) // 128
            w3r = np.asarray(mp['w3'], F32).reshape(K, din, dout).reshape(C, dout)
            b3r = np.asarray(mp['b3'], F32).reshape(din, dout)
            w3e = np.concatenate([w3r, b3r], 0)       # [C+din, dout]
            w3p = np.zeros((ncb * 128, dout), F32)
            w3p[:rows] = w3e
            # SBUF layout [128, ncb*dout]: sb[p, cb*dout + j] = w3p[cb*128 + p, j]
            w3sb = w3p.reshape(ncb, 128, dout).transpose(1, 0, 2).reshape(128, ncb * dout)
            shared[f'w3sb_{tag}'] = _bf(w3sb)
            shared[f'root_{tag}'] = _bf(cp['root'])   # [din, dout]
            shared[f'cbias_{tag}'] = np.ascontiguousarray(
                np.asarray(cp['bias'], F32)[:, None])  # [dout, 1] f32

    shared['lw1'] = _bf(params['lin1']['w'])          # [128, 128]
    shared['lb1'] = np.ascontiguousarray(np.asarray(params['lin1']['b'], F32)[:, None])
    shared['lw2'] = _bf(params['lin2']['w'])          # [128, 64]
    shared['lb2'] = np.ascontiguousarray(np.asarray(params['lin2']['b'], F32)[:, None])
    shared['lw3'] = _bf(params['lin3']['w'])          # [64, 64]
    shared['lb3'] = np.ascontiguousarray(np.asarray(params['lin3']['b'], F32)[:, None])
    wout = np.asarray(params['out']['w'], F32)        # [128, 1]
    shared['wo1'] = _bf(wout[:64])
    shared['wo2'] = _bf(wout[64:])
    shared['bout'] = np.ascontiguousarray(np.asarray(params['out']['b'], F32).reshape(1, 1))
    shared['identity'] = _bf(np.eye(128, dtype=F32))
    shared['ones'] = _bf(np.ones((1, EB), F32))

    # --- per-core tensors -------------------------------------------------
    in_maps = []
    for c in range(NCORES):
        m = dict(shared)
        nb0 = c * NPC
        eb0 = c * EPC
        xs = x[nb0:nb0 + NPC]                          # [320, 4]
        # gather rhs layout: hblk [80, ET*din]
        m['h0'] = _bf(xs.reshape(ET, NB, 4).transpose(1, 0, 2).reshape(NB, ET * 4))
        m['hT0'] = _bf(xs.T)                           # [4, 320]
        for b in range(2):
            ea = eattrs[b][eb0:eb0 + EPC]              # [512, 3]
            et = np.concatenate([ea.T, np.ones((1, EPC), F32)], 0)  # [4, 512]
            m[f'eatT_{b}'] = _bf(et)
        G = np.zeros((NB, EPC), F32)
        D = np.zeros((EB, ET * NB), F32)
        for j in range(EPC):
            e = eb0 + j
            et = j // EB
            sl = src[e] - (nb0 + et * NB)
            dl = dst[e] - (nb0 + et * NB)
            assert 0 <= sl < NB and 0 <= dl < NB, (
                "edges must stay within their 8-graph block")
            G[sl, j] = 1.0
            D[j % EB, et * NB + dl] = 1.0
        m['G'] = _bf(G)
        m['D'] = _bf(D)
        in_maps.append(m)
    return in_maps


def _build_nc(loop_n=1):
    import concourse.bacc as bacc
    import concourse.bass as bass
    import concourse.mybir as mybir
    import concourse.tile as tile
    from contextlib import ExitStack, nullcontext

    bf16 = mybir.dt.bfloat16
    f32 = mybir.dt.float32
    AF = mybir.ActivationFunctionType
    MUL = mybir.AluOpType.mult

    nc = bacc.Bacc(
        "TRN2",
        target_bir_lowering=False,
        debug=False,
        enable_asserts=False,
        num_devices=NCORES,
    )

    din0 = {}

    def dram(name, shape, dt=bf16, kind="ExternalInput"):
        t = nc.dram_tensor(name, list(shape), dt, kind=kind)
        din0[name] = t.ap()
        return din0[name]

    # inputs
    for b in range(2):
        for l, (din, dout) in enumerate(DIMS):
            tag = f"{b}_{l}"
            C = K * din
            ncb = (C + din + 127) // 128
            dram(f'w1e_{tag}', (4, K))
            dram(f'w2d_{tag}', (K, 2 * K))
            dram(f'b2d_{tag}', (1, 2 * K))
            dram(f'w3sb_{tag}', (128, ncb * dout))
            dram(f'root_{tag}', (din, dout))
            dram(f'cbias_{tag}', (dout, 1), f32)
        dram(f'eatT_{b}', (4, EPC))
    dram('lw1', (128, 128)); dram('lb1', (128, 1), f32)
    dram('lw2', (128, 64)); dram('lb2', (64, 1), f32)
    dram('lw3', (64, 64)); dram('lb3', (64, 1), f32)
    dram('wo1', (64, 1)); dram('wo2', (64, 1)); dram('bout', (1, 1), f32)
    dram('identity', (128, 128))
    dram('ones', (1, EB))
    dram('h0', (NB, ET * 4))
    dram('hT0', (4, NPC))
    dram('G', (NB, EPC))
    dram('D', (EB, ET * NB))
    y_dram = dram('y', (1, GPC), f32, kind="ExternalOutput")

    with tile.TileContext(nc) as tc, ExitStack() as ctx:
        cpool = ctx.enter_context(tc.tile_pool(name="consts", bufs=1))
        wpool = ctx.enter_context(tc.tile_pool(name="weights", bufs=2))
        apool = ctx.enter_context(tc.tile_pool(name="acts", bufs=2))
        zpool = ctx.enter_context(tc.tile_pool(name="zs", bufs=1))
        pp_mid = ctx.enter_context(tc.tile_pool(name="pmid", bufs=1, space="PSUM"))
        pp_sml = ctx.enter_context(tc.tile_pool(name="psml", bufs=1, space="PSUM"))
        pp_st = ctx.enter_context(tc.tile_pool(name="pst", bufs=2, space="PSUM"))
        pp_agg = ctx.enter_context(tc.tile_pool(name="pagg", bufs=1, space="PSUM"))

        def load_const(name, shape=None, dt=bf16):
            ap = din0[name]
            t = cpool.tile(list(ap.shape), dt, name=f"sb_{name}", tag=f"c_{name}")
            nc.sync.dma_start(out=t[:, :], in_=ap[:, :])
            return t

        consts = {}
        for name, ap in list(din0.items()):
            if name in ('y',):
                continue
            if name.startswith(('cbias', 'lb', 'bout')):
                consts[name] = load_const(name, dt=f32)
            elif name.startswith('w3sb'):
                continue  # loaded per-conv (double-buffered)
            else:
                consts[name] = load_const(name)

        x3T = cpool.tile([128, NPC], bf16, tag="x3T", name="x3T")
        loop_cm = tc.For_i(0, loop_n, 1) if loop_n > 1 else nullcontext()
        ctx.enter_context(loop_cm)
        for b in range(2):
            hT = consts['hT0']        # [din, 320] bf16 (partitions = din)
            hblk = consts['h0']       # [80, ET*din]
            for l, (din, dout) in enumerate(DIMS):
                tag = f"{b}_{l}"
                C = K * din
                rows = C + din
                ncb = (rows + 127) // 128
                pad = ncb * 128 - rows

                w3sb = wpool.tile([128, ncb * dout], bf16, tag="w3sb", name=f"w3sb_t_{tag}")
                nc.sync.dma_start(out=w3sb[:, :], in_=din0[f'w3sb_{tag}'][:, :])

                # ---- edge MLP layer 1: midT [64, 512] ----
                mid_ps = pp_mid.tile([K, EPC], f32, tag="midps", name=f"midps_{tag}")
                nc.tensor.matmul(mid_ps[:, :], consts[f'w1e_{tag}'][:, :],
                                 consts[f'eatT_{b}'][:, :])
                midT = apool.tile([K, EPC], bf16, tag="midT", name=f"midT_{tag}")
                nc.scalar.activation(midT[:, :], mid_ps[:, :], AF.Relu)

                # ---- per edge-tile: h2d, h_src gather, z build ----
                zts = []
                for et in range(ET):
                    h2_ps = pp_sml.tile([EB, 2 * K], f32, tag="h2ps", name=f"h2ps_{tag}_{et}")
                    nc.tensor.matmul(h2_ps[:, :], midT[:, et * EB:(et + 1) * EB],
                                     consts[f'w2d_{tag}'][:, :], start=True, stop=False)
                    nc.tensor.matmul(h2_ps[:, :], consts['ones'][:, :],
                                     consts[f'b2d_{tag}'][:, :], start=False, stop=True)
                    h2d = apool.tile([EB, 2 * K], bf16, tag="h2d", name=f"h2d_{tag}_{et}")
                    nc.scalar.activation(h2d[:, :], h2_ps[:, :], AF.Relu)

                    hs_ps = pp_sml.tile([EB, din], f32, tag="hsps", name=f"hsps_{tag}_{et}")
                    nc.tensor.matmul(hs_ps[:, :], consts['G'][:, et * EB:(et + 1) * EB],
                                     hblk[:, et * din:(et + 1) * din])
                    hs = apool.tile([EB, din], bf16, tag="hs", name=f"hs_{tag}_{et}")
                    nc.scalar.copy(hs[:, :], hs_ps[:, :])

                    zt = zpool.tile([EB, ncb * 128], bf16, tag=f"z{et}",
                                    name=f"z_{tag}_{et}")
                    # z[e, k*din + i] = h2[e,k] * hs[e,i] via paired-bf16 APs
                    d2 = din // 2
                    pstep_z = zt.ap[0][0]
                    out_ap = bass.AP(zt.tensor, zt.offset,
                                     [zt.ap[0], [din, K], [2, d2], [1, 2]])
                    in0_ap = bass.AP(h2d.tensor, h2d.offset,
                                     [h2d.ap[0], [2, K], [0, d2], [1, 2]])
                    in1_ap = bass.AP(hs.tensor, hs.offset,
                                     [hs.ap[0], [0, K], [2, d2], [1, 2]])
                    nc.vector.tensor_tensor(out_ap, in0_ap, in1_ap, MUL)
                    # bias block: copy h_src into z[:, C:C+din]
                    nc.vector.tensor_copy(zt[:, C:C + din], hs[:, :])
                    if pad:
                        nc.vector.memset(zt[:, rows:], 0.0)
                    zts.append(zt)

                # ---- stage 1 + stage 2 per contraction block ----
                agg_ps = pp_agg.tile([dout, NPC], f32, tag="aggps", name=f"aggps_{tag}")
                for cb in range(ncb):
                    st_ps = pp_st.tile([128, NPC], f32, tag="stps",
                                       name=f"stps_{tag}_{cb}")
                    for et in range(ET):
                        nc.tensor.matmul(
                            st_ps[:, et * NB:(et + 1) * NB],
                            zts[et][:, cb * 128:(cb + 1) * 128],
                            consts['D'][:, et * NB:(et + 1) * NB],
                            start=True, stop=True, skip_group_check=(et > 0))
                    st_sb = apool.tile([128, NPC], bf16, tag="stsb",
                                       name=f"stsb_{tag}_{cb}")
                    if cb % 2 == 0:
                        nc.scalar.copy(st_sb[:, :], st_ps[:, :])
                    else:
                        nc.vector.tensor_copy(st_sb[:, :], st_ps[:, :])
                    nc.tensor.matmul(agg_ps[:, :], w3sb[:, cb * dout:(cb + 1) * dout],
                                     st_sb[:, :], start=(cb == 0), stop=False)
                # root term
                nc.tensor.matmul(agg_ps[:, :], consts[f'root_{tag}'][:, :],
                                 hT[:din, :], start=False, stop=True)

                # ---- relu + bias -> new hT ----
                hT_new = apool.tile([128, NPC], bf16, tag="hT", name=f"hT_{tag}")
                nc.scalar.activation(hT_new[:dout, :], agg_ps[:, :], AF.Relu,
                                     bias=consts[f'cbias_{tag}'][:, :])
                hT = hT_new[:dout, :]

                # ---- transpose for next layer's gather ----
                if l < 3:
                    hblk_new = apool.tile([NB, ET * dout], bf16, tag="hblk",
                                          name=f"hblk_{tag}")
                    for et in range(ET):
                        tp_ps = pp_sml.tile([NB, dout], bf16, tag="tpps",
                                            name=f"tpps_{tag}_{et}")
                        nc.tensor.transpose(tp_ps[:, :], hT[:, et * NB:(et + 1) * NB],
                                            consts['identity'][:dout, :dout])
                        nc.scalar.copy(hblk_new[:, et * dout:(et + 1) * dout],
                                       tp_ps[:, :])
                    hblk = hblk_new

            nc.scalar.copy(x3T[b * 64:(b + 1) * 64, :], hT)

        # ---- head: concat -> lin1 -> lin2 -> lin3 -> readout ----
        l1_ps = pp_st.tile([128, NPC], f32, tag="stps", name="l1ps")
        nc.tensor.matmul(l1_ps[:, :], consts['lw1'][:, :], x3T[:, :])
        l1T = apool.tile([128, NPC], bf16, tag="hT", name="l1T")
        nc.scalar.activation(l1T[:, :], l1_ps[:, :], AF.Identity,
                             bias=consts['lb1'][:, :])

        l2_ps = pp_st.tile([64, NPC], f32, tag="stps", name="l2ps")
        nc.tensor.matmul(l2_ps[:, :], consts['lw2'][:, :], l1T[:, :])
        l2T = apool.tile([64, NPC], bf16, tag="hblk", name="l2T")
        nc.scalar.activation(l2T[:, :], l2_ps[:, :], AF.Identity,
                             bias=consts['lb2'][:, :])

        l3_ps = pp_st.tile([64, NPC], f32, tag="stps", name="l3ps")
        nc.tensor.matmul(l3_ps[:, :], consts['lw3'][:, :], l2T[:, :])
        l3T = apool.tile([64, NPC], bf16, tag="hT", name="l3T")
        nc.scalar.activation(l3T[:, :], l3_ps[:, :], AF.Identity,
                             bias=consts['lb3'][:, :])

        # readout: y[g] = wo1 . l3T[:, 10g] + wo2 . l3T[:, 10g+1] + bout
        y_ps = pp_sml.tile([1, GPC], f32, tag="h2ps", name="yps")
        rhs0 = bass.AP(l3T.tensor, l3T.offset, [l3T.ap[0], [NPG, GPC]])
        rhs1 = bass.AP(l3T.tensor, l3T.offset + 1, [l3T.ap[0], [NPG, GPC]])
        nc.tensor.matmul(y_ps[:, :], consts['wo1'][:, :], rhs0, start=True, stop=False)
        nc.tensor.matmul(y_ps[:, :], consts['wo2'][:, :], rhs1, start=False, stop=True)
        y_sb = apool.tile([1, GPC], f32, tag="ysb", name="ysb")
        nc.scalar.activation(y_sb[:, :], y_ps[:, :], AF.Identity,
                             bias=consts['bout'][:, :])
        nc.sync.dma_start(out=y_dram[:, :], in_=y_sb[:, :])

    nc.compile()
    return nc


_NC_CACHE = {}


def _get_nc(loop_n=1):
    if loop_n not in _NC_CACHE:
        _NC_CACHE[loop_n] = _build_nc(loop_n)
    return _NC_CACHE[loop_n]


def kernel(x, edge_index, edge_attr1, edge_attr2, batch, params):
    from concourse.bass_utils import run_bass_kernel_spmd
    in_maps = _host_prep(x, edge_index, edge_attr1, edge_attr2, batch, params)
    nc = _get_nc()
    res = run_bass_kernel_spmd(nc, in_maps, core_ids=list(range(NCORES)))
    y = np.concatenate([res.results[c]['y'].reshape(GPC, 1)
                        for c in range(NCORES)], 0)
    return y.astype(F32)
